# revision 3
# baseline (speedup 1.0000x reference)
"""Linear-chain CRF negative mean log-likelihood on 8 Trainium2 NeuronCores.

Full inputs in, full (scalar) output out. Data-parallel over the batch: each
core processes B/8 = 1024 sequences end-to-end.

v3 architecture (per core), engine-balanced around the serial forward-DP:
  - emission scores em[32g+l, b'] per step via 4 matmuls: group 0 rides the
    fp8 DoubleRow perf mode (0.5 cy/col; only legal at PSUM partition 0),
    groups 1-3 plain fp8. x is host-marshalled into the matching layouts.
  - partition function via the exp-space forward DP
    A_t = (expBD^T A_{t-1}) o exp(em_t - c_t), split into two 128-column
    half-chains so the PE->DVE->PE dependency cycle (53 + ~104 + 258 + ~81 ns)
    stays under the engine budgets. The DVE runs ONLY these two A-multiplies
    per step - all gold-score work is off the DVE.
  - gold emission score: Pool computes P_t = OHT_t o E_t (the only
    PSUM-free elementwise engine); a ones-gather matmul with a slot-shifted
    stationary accumulates s[4s+g, b'] = E_t[y_t, b] into a rotating PSUM
    block (8 steps per block); Act copies blocks out and the host does the
    ln + sum (the c-schedule cancels against logZ).
  - gold transition score: paired count matmuls anchored at even t with
    moving [oh_{t-1} | oh_{t+1}] in fp8 DoubleRow (26 cy each, 4/step avg)
    accumulate [l_t, l_prev | l_next] counts; host contracts with Tr^T | Tr.
  - logZ: group sums zs = onesBD^T A_63 shipped raw; host ln + reduce.
  - 8 warmup matmuls hold the PE p-state ramp; DMAs are batched into large
    chunks ordered so each step's x / onehot data lands ahead of use.

Each core writes partial tensors; the host combines them into the loss.
"""

import numpy as np

L = 26
D = 128
T = 64
B = 8192
NCORES = 8
BC = B // NCORES  # 1024 sequences per core

# Per-step scale schedule for the exp-space forward DP (subtracted from em at
# step t so the running A stays well inside fp32 range). It cancels exactly
# in the host finale (gold ln-sum and logZ shift by the same B*sum(C_SCHED)).
C_SCHED = np.array([
    0.933700, 3.577268, 3.746262, 4.537820, 4.040299, 4.041378, 4.067604, 4.107736,
    4.101158, 4.091968, 3.790887, 4.203616, 4.050755, 4.272369, 3.625527, 3.864683,
    4.922722, 4.424649, 3.161501, 4.352942, 3.777887, 4.534618, 4.044740, 3.829787,
    4.015547, 4.710327, 3.921810, 4.398400, 4.176108, 3.293104, 4.761852, 3.388780,
    3.782803, 4.950686, 3.611373, 4.506680, 3.005395, 4.511179, 3.714007, 4.567758,
    3.993558, 4.003791, 4.249708, 4.211322, 4.069564, 4.249093, 3.763951, 3.601156,
    5.005219, 3.880518, 4.270474, 3.819207, 3.979380, 4.438228, 4.122883, 2.404448,
    4.026374, 5.060853, 4.290274, 4.044138, 3.681486, 4.656340, 3.408876, 3.532320,
], dtype=np.float64)

_CACHE: dict = {}
TRACE = False  # set by test harness to capture NTFF profile / exec time

# Instruction opcodes whose hardware structs tolerate multiple sync waits (or
# that walrus lowers specially). Everything else gets excess waits peeled onto
# EventSemaphore instructions inserted just before it (same engine).
_MULTIWAIT_OK = {
    "Call",
    "UnconditionalBranch",
    "ConditionalBranch",
}


def _legalize_waits(bir_bytes: bytes) -> bytes:
    """Split >1 sync waits per compute instruction into EventSemaphore preludes.

    The TRN2 64-byte instruction structs hold a single sync-wait command;
    Tile attaches multi-engine waits directly, which walrus codegen rejects
    ("Too many sync wait commands"). Peeling extra waits onto same-engine
    EventSemaphore instructions placed immediately before is semantically
    identical (engine streams execute in order).
    """
    import json

    d = json.loads(bir_bytes)
    n = 0
    for fn in d["functions"]:
        for blk in fn["blocks"]:
            out = []
            for inst in blk["instructions"]:
                si = inst.get("sync_info")
                if (
                    si
                    and len(si.get("on_wait", [])) > 1
                    and inst["opcode"] not in _MULTIWAIT_OK
                ):
                    waits = si["on_wait"]
                    for w in waits[:-1]:
                        n += 1
                        out.append({
                            "debug": inst.get("debug", 0),
                            "engine": inst["engine"],
                            "ins": [],
                            "name": f"wsplit-{n}-{inst['name']}",
                            "opcode": "EventSemaphore",
                            "outs": [],
                            "sync_info": {"on_update": [], "on_wait": [w]},
                        })
                    si["on_wait"] = [waits[-1]]
                out.append(inst)
            blk["instructions"] = out
    return json.dumps(d).encode()


# cblob byte layout (per partition)
_CB_WDR = 0        # [0:64)    Wdr fp8 [64 part, 2, 32]  DoubleRow weights
_CB_WT = 64        # [64:96)   Wt32 fp8 [128, 32]        plain em weights (W^T)
_CB_EXPBD = 96     # [96:352)  expBD bf16 [128, 128]     block-diag exp(Tr)
_CB_CBIAS = 352    # [352:608) cbias f32 [128, 64]       -C_SCHED broadcast
_CB_ONESSH = 608   # [608:1120) onesSh bf16 [128, 8, 32] slot-shifted gather
_CB_ONESBD = 1120  # [1120:1128) onesBD bf16 [128, 4]    group-sum mask
_CB_END = 1128

# out tensor column layout (f32 [128, 2368])
_OUT_S = 0         # rows 0:32, cols [0:2048)   s blocks [32, 8, 256]
_OUT_ZS = 2048     # rows 0:4,  cols [2048:2304) zs [4, 256]
_OUT_CC = 2304     # rows 0:26, cols [2304:2356) CC [26, 52]
_OUT_COLS = 2368


def build_program():
    """Build the per-core Bass/Tile program (identical SPMD program)."""
    from contextlib import ExitStack

    import concourse.bass as bass
    import concourse.tile as tile
    from concourse import mybir

    f32 = mybir.dt.float32
    bf16 = mybir.dt.bfloat16
    f8 = mybir.dt.float8e4
    AF = mybir.ActivationFunctionType
    OP = mybir.AluOpType
    DR = mybir.MatmulPerfMode.DoubleRow

    nc = bass.Bass("TRN2", target_bir_lowering=False, debug=False)

    xa_d = nc.dram_tensor("xa", [64, 2, T, 256], f8, kind="ExternalInput").ap()
    xb_d = nc.dram_tensor("xb", [D, T, 768], f8, kind="ExternalInput").ap()
    oht_d = nc.dram_tensor("oht", [128, T, 256], f8, kind="ExternalInput").ap()
    ohdr_d = nc.dram_tensor("ohdr", [64, 2, T, 8, L], f8, kind="ExternalInput").ap()
    c_d = nc.dram_tensor("cst", [128, _CB_END], mybir.dt.uint8, kind="ExternalInput").ap()
    out_d = nc.dram_tensor("out", [128, _OUT_COLS], f32, kind="ExternalOutput").ap()

    with ExitStack() as ctx:
        tc = ctx.enter_context(tile.TileContext(nc))

        const = ctx.enter_context(tc.tile_pool(name="const", bufs=1))
        epool = ctx.enter_context(tc.tile_pool(name="epool", bufs=4))
        apool = ctx.enter_context(tc.tile_pool(name="apool", bufs=2))
        ppool = ctx.enter_context(tc.tile_pool(name="ppool", bufs=2))
        fpool = ctx.enter_context(tc.tile_pool(name="fpool", bufs=1))
        ps_em = ctx.enter_context(tc.tile_pool(name="ps_em", bufs=2, space="PSUM"))
        ps_u = ctx.enter_context(tc.tile_pool(name="ps_u", bufs=2, space="PSUM"))
        ps_sg = ctx.enter_context(tc.tile_pool(name="ps_sg", bufs=2, space="PSUM"))
        ps_cc = ctx.enter_context(tc.tile_pool(name="ps_cc", bufs=1, space="PSUM"))

        # ---- PE p-state warmup: dummy matmuls keep the tensor engine's
        # ramp running so the first real emissions hit full clock
        wz = const.tile([128, 416], bf16)
        nc.vector.memset(wz, 0.0)
        for w in range(8):
            wps = ps_em.tile([128, 256], f32, tag="em", name="warm")
            nc.tensor.matmul(
                wps, lhsT=wz[:, 0:128], rhs=wz[:, 0:256], start=True, stop=True
            )

        # ---- SBUF input tiles ----
        xa = const.tile([64, 2, T, 256], f8)
        xb = const.tile([D, T, 768], f8)
        oht = const.tile([128, T, 256], f8)
        ohdr = const.tile([64, 2, T, 8, L], f8)
        cblob = const.tile([128, _CB_END], mybir.dt.uint8)
        s_sb = fpool.tile([32, 8, 256], f32)

        # packed constants first: single small DMA gates everything
        nc.scalar.dma_start(out=cblob, in_=c_d)

        def dma_xa(t0, t1):
            nc.sync.dma_start(out=xa[:, :, t0:t1, :], in_=xa_d[:, :, t0:t1, :])

        def dma_xb(t0, t1):
            nc.sync.dma_start(out=xb[:, t0:t1, :], in_=xb_d[:, t0:t1, :])

        def dma_oht(t0, t1):
            nc.sync.dma_start(out=oht[:, t0:t1, :], in_=oht_d[:, t0:t1, :])

        def dma_ohdr(t0, t1):
            nc.sync.dma_start(
                out=ohdr[:, :, t0:t1, :, :], in_=ohdr_d[:, :, t0:t1, :, :]
            )

        # front-load the first few steps, then stream large chunks
        dma_xa(0, 4)
        dma_xb(0, 4)
        dma_oht(0, 4)
        dma_ohdr(0, 4)
        dma_xa(4, 12)
        dma_xb(4, 12)
        dma_oht(4, 12)
        dma_ohdr(4, 12)
        dma_xa(12, 24)
        dma_xb(12, 24)
        dma_oht(12, 24)
        dma_ohdr(12, 24)
        dma_xa(24, 40)
        dma_xb(24, 40)
        dma_oht(24, 40)
        dma_ohdr(24, 40)
        dma_xa(40, 64)
        dma_xb(40, 64)
        dma_oht(40, 64)
        dma_ohdr(40, 64)

        # ---- bitcast views into the packed constant blob ----
        Wdr = cblob[0:64, _CB_WDR : _CB_WDR + 64].bitcast(f8).rearrange(
            "p (j m) -> p j m", j=2
        )
        Wt32 = cblob[:, _CB_WT : _CB_WT + 32].bitcast(f8)
        expBD = cblob[:, _CB_EXPBD : _CB_EXPBD + 256].bitcast(bf16)
        cbias = cblob[:, _CB_CBIAS : _CB_CBIAS + 256].bitcast(f32)
        onesSh = cblob[:, _CB_ONESSH : _CB_ONESSH + 512].bitcast(bf16).rearrange(
            "p (s m) -> p s m", s=8
        )
        onesBD = cblob[:, _CB_ONESBD : _CB_ONESBD + 8].bitcast(bf16)

        # persistent psum accumulator for paired transition counts
        CC_ps = ps_cc.tile([L, 2 * L], f32)
        nc.vector.memset(CC_ps, 0.0)

        E_t = {}
        em_t = {}

        def emit_em(t):
            em_ps = ps_em.tile([128, 256], f32, tag="em")
            em_t[t] = em_ps
            # group 0: fp8 DoubleRow (only legal at psum partition 0)
            nc.tensor.matmul(
                em_ps[0:32, :],
                lhsT=Wdr,
                rhs=xa[:, :, t, :],
                start=True,
                stop=True,
                perf_mode=DR,
                tile_position=(0, 0),
            )
            # groups 1-3: plain fp8
            for g in range(1, 4):
                nc.tensor.matmul(
                    em_ps[32 * g : 32 * (g + 1), :],
                    lhsT=Wt32,
                    rhs=xb[:, t, 256 * (g - 1) : 256 * g],
                    start=True,
                    stop=True,
                    tile_position=(0, 32 * g),
                )

        def emit_exp(t):
            E = epool.tile([128, 256], bf16, tag="E", name="E")
            nc.scalar.activation(
                E, em_t.pop(t), AF.Exp, bias=cbias[:, t : t + 1], scale=1.0
            )
            E_t[t] = E

        sg_tiles = {}

        def emit_gather(t):
            # P_t = OHT_t o E_t on Pool (SBUF-only engine), then the
            # slot-shifted ones-gather accumulates E_t[y_t, b] into psum
            P = ppool.tile([128, 256], bf16, tag="P", name="P")
            nc.gpsimd.tensor_tensor(out=P, in0=oht[:, t, :], in1=E_t[t], op=OP.mult)
            s, q = t % 8, t // 8
            if s == 0:
                sg_tiles[q % 2] = ps_sg.tile([32, 256], f32, tag="sg", name="sg")
            nc.tensor.matmul(
                sg_tiles[q % 2],
                lhsT=onesSh[:, s, :],
                rhs=P,
                start=(s == 0),
                stop=(s == 7),
            )

        def emit_scopy(q):
            nc.scalar.copy(s_sb[:, q, :], sg_tiles[q % 2])
            if q % 2 == 1:
                nc.sync.dma_start(
                    out=out_d[0:32, 256 * (q - 1) : 256 * (q + 1)],
                    in_=s_sb[:, q - 1 : q + 1, :].rearrange("p a b -> p (a b)"),
                )

        def emit_counts(a):
            # paired transition counts, anchor a (even): one fp8 DoubleRow
            # matmul per b-chunk covers pairs (a-1,a) [transposed] and (a,a+1)
            for c in range(8):
                lhsT = ohdr[:, :, a, c, :]
                if a == 0:
                    rhs = ohdr[:, :, 1:2, c, :]
                    outap = CC_ps[:, L : 2 * L]
                else:
                    rhs = ohdr[:, :, a - 1 : a + 2 : 2, c, :]
                    outap = CC_ps
                nc.tensor.matmul(
                    outap,
                    lhsT=lhsT,
                    rhs=rhs,
                    start=False,
                    stop=False,
                    perf_mode=DR,
                    skip_group_check=True,
                )

        # ---- software-pipelined main loop ----
        emit_em(0)
        emit_exp(0)
        emit_em(1)
        emit_exp(1)
        A_prev = None
        for t in range(T):
            E = E_t[t]
            if t == 0:
                A_prev = E
            else:
                with tc.high_priority(offset=60):
                    u_ps = ps_u.tile([128, 256], f32, tag="u")
                    A_new = apool.tile([128, 256], bf16, tag="A", name="A")
                    for h in range(2):
                        cs = slice(128 * h, 128 * (h + 1))
                        nc.tensor.matmul(
                            u_ps[:, cs],
                            lhsT=expBD,
                            rhs=A_prev[:, cs],
                            start=True,
                            stop=True,
                        )
                        nc.vector.tensor_mul(A_new[:, cs], u_ps[:, cs], E[:, cs])
                    A_prev = A_new
            if t + 2 < T:
                emit_em(t + 2)
                emit_exp(t + 2)
            emit_gather(t)
            E_t.pop(t)
            if t % 8 == 7:
                emit_scopy(t // 8)
            if t >= 2 and t % 2 == 0:
                emit_counts(t - 2)
        emit_counts(T - 2)

        # ---- finale ----
        zs_ps = ps_em.tile([4, 256], f32, tag="em", name="zs")
        nc.tensor.matmul(zs_ps, lhsT=onesBD, rhs=A_prev, start=True, stop=True)
        zs_sb = fpool.tile([4, 256], f32)
        nc.scalar.copy(zs_sb, zs_ps)
        nc.scalar.dma_start(out=out_d[0:4, _OUT_ZS : _OUT_ZS + 256], in_=zs_sb)
        cc_sb = fpool.tile([L, 2 * L], f32)
        nc.scalar.copy(cc_sb, CC_ps)
        nc.scalar.dma_start(out=out_d[0:L, _OUT_CC : _OUT_CC + 2 * L], in_=cc_sb)

    fixed = _legalize_waits(nc.to_json_bytes())
    nc.to_json_bytes = lambda: fixed  # shadow for all compile paths
    return nc


def _marshal(feat_x, input_y, params):
    """Host-side input marshalling: dtype casts + layout transposes/onehots."""
    import ml_dtypes

    f8 = ml_dtypes.float8_e4m3
    bf16 = ml_dtypes.bfloat16

    feat_x = np.asarray(feat_x, dtype=np.float32)
    input_y = np.asarray(input_y, dtype=np.int32)
    params = np.asarray(params, dtype=np.float32)

    W = params[: L * D].reshape(L, D)
    Tr = params[L * D :].reshape(L, L).astype(np.float64)

    # ---- packed per-partition constants ----
    cblob = np.zeros((128, _CB_END), dtype=np.uint8)
    # Wdr [64, 2, 32]: Wdr[k, j, m] = W[m, 2k+j]
    wdr = np.zeros((64, 2, 32), dtype=np.float32)
    wdr[:, :, :L] = W.T.reshape(64, 2, L)
    cblob[0:64, _CB_WDR : _CB_WDR + 64] = (
        wdr.astype(f8).view(np.uint8).reshape(64, 64)
    )
    # Wt32 [128, 32]: W^T zero-padded
    wt32 = np.zeros((D, 32), dtype=np.float32)
    wt32[:, :L] = W.T
    cblob[:, _CB_WT : _CB_WT + 32] = wt32.astype(f8).view(np.uint8)
    # expBD block-diag exp(Tr)
    expbd = np.zeros((128, 128), dtype=np.float32)
    for g in range(4):
        expbd[32 * g : 32 * g + L, 32 * g : 32 * g + L] = np.exp(Tr)
    cblob[:, _CB_EXPBD : _CB_EXPBD + 256] = expbd.astype(bf16).view(np.uint8)
    # cbias
    cbias = np.tile(-C_SCHED.astype(np.float32), (128, 1))
    cblob[:, _CB_CBIAS : _CB_CBIAS + 256] = cbias.view(np.uint8)
    # onesSh [128, 8, 32]: onesSh[32g+l, s, 4s+g] = 1 for l < L
    onessh = np.zeros((128, 8, 32), dtype=np.float32)
    for g in range(4):
        for s in range(8):
            onessh[32 * g : 32 * g + L, s, 4 * s + g] = 1.0
    cblob[:, _CB_ONESSH : _CB_ONESSH + 512] = (
        onessh.astype(bf16).view(np.uint8).reshape(128, 512)
    )
    # onesBD [128, 4]
    onesbd = np.zeros((128, 4), dtype=np.float32)
    for g in range(4):
        onesbd[32 * g : 32 * g + L, g] = 1.0
    cblob[:, _CB_ONESBD : _CB_ONESBD + 8] = onesbd.astype(bf16).view(np.uint8)
    cblob = np.ascontiguousarray(cblob)

    # x transposed once: xT[d, t, b]
    xT = np.ascontiguousarray(feat_x.transpose(2, 1, 0)).astype(f8)

    in_maps = []
    for m in range(NCORES):
        sl = slice(m * BC, (m + 1) * BC)
        xm = xT[:, :, sl]  # [128, T, 1024] fp8
        ym = input_y[sl]  # [1024, T]
        # xa [64, 2, T, 256]: group 0 (b 0:256), d = 2k+j
        xam = np.ascontiguousarray(
            xm[:, :, 0:256].reshape(64, 2, T, 256)
        )
        # xb [128, T, 768]: groups 1-3 (b 256:1024)
        xbm = np.ascontiguousarray(xm[:, :, 256:1024])
        # oht [128, T, 256]: oht[32g+l, t, b'] = (y[256g+b', t] == l)
        lab = ym.reshape(4, 256, T)  # [g, b', t]
        lvec = np.arange(32)
        ohtm = (
            lab[:, None, :, :] == lvec[None, :, None, None]
        )  # [g, l(32), b', t]
        ohtm = np.ascontiguousarray(
            ohtm.reshape(128, 256, T).transpose(0, 2, 1).astype(np.float32)
        ).astype(f8)
        # ohdr [64, 2, T, 8, L]: ohdr[k, j, t, c, l] = (y[128c+2k+j, t] == l)
        labc = ym.reshape(8, 64, 2, T)  # [c, k, j, t]
        ohdrm = labc[:, :, :, :, None] == np.arange(L)[None, None, None, None, :]
        ohdrm = np.ascontiguousarray(
            ohdrm.transpose(1, 2, 3, 0, 4).astype(np.float32)
        ).astype(f8)
        in_maps.append(
            {"xa": xam, "xb": xbm, "oht": ohtm, "ohdr": ohdrm, "cst": cblob}
        )
    return in_maps


def kernel(feat_x: np.ndarray, input_y: np.ndarray, params: np.ndarray) -> np.ndarray:
    from concourse.bass_utils import run_bass_kernel_spmd

    if "nc" not in _CACHE:
        _CACHE["nc"] = build_program()
    nc = _CACHE["nc"]

    in_maps = _marshal(feat_x, input_y, params)

    res = run_bass_kernel_spmd(
        nc, in_maps, core_ids=list(range(NCORES)), trace=TRACE
    )
    _CACHE["last_results"] = res

    params = np.asarray(params, dtype=np.float64)
    Tr = params[L * D :].reshape(L, L)

    lns_sum = lnz_sum = tr_sum = 0.0
    for m in range(NCORES):
        out = res.results[m]["out"].astype(np.float64)
        s = out[0:32, _OUT_S : _OUT_S + 2048]
        lns_sum += np.log(np.maximum(s, 1e-300)).sum()
        zs = out[0:4, _OUT_ZS : _OUT_ZS + 256]
        lnz_sum += np.log(zs).sum()
        cc = out[0:L, _OUT_CC : _OUT_CC + 2 * L]
        tr_sum += (Tr.T * cc[:, 0:L]).sum() + (Tr * cc[:, L : 2 * L]).sum()
    loss = -(lns_sum + tr_sum - lnz_sum) / B
    return np.float32(loss)


# revision 35
# speedup vs baseline: 1.3426x; 1.3426x over previous
"""Linear-chain CRF negative mean log-likelihood on 8 Trainium2 NeuronCores.

Full inputs in, full (scalar) output out. Data-parallel over the batch: each
core processes B/8 = 1024 sequences end-to-end.

v3 architecture (per core), engine-balanced around the serial forward-DP:
  - emission scores em[32g+l, b'] per step via 4 matmuls: group 0 rides the
    fp8 DoubleRow perf mode (0.5 cy/col; only legal at PSUM partition 0),
    groups 1-3 plain fp8. x is host-marshalled into the matching layouts.
  - partition function via the exp-space forward DP
    A_t = (expBD^T A_{t-1}) o exp(em_t - c_t), split into two 128-column
    half-chains so the PE->DVE->PE dependency cycle (53 + ~104 + 258 + ~81 ns)
    stays under the engine budgets. The DVE runs ONLY these two A-multiplies
    per step - all gold-score work is off the DVE.
  - gold emission score: Pool computes P_t = OHT_t o E_t (the only
    PSUM-free elementwise engine); a ones-gather matmul with a slot-shifted
    stationary accumulates s[4s+g, b'] = E_t[y_t, b] into a rotating PSUM
    block (8 steps per block); Act copies blocks out and the host does the
    ln + sum (the c-schedule cancels against logZ).
  - gold transition score: paired count matmuls anchored at even t with
    moving [oh_{t-1} | oh_{t+1}] in fp8 DoubleRow (26 cy each, 4/step avg)
    accumulate [l_t, l_prev | l_next] counts; host contracts with Tr^T | Tr.
  - logZ: group sums zs = onesBD^T A_63 shipped raw; host ln + reduce.
  - 8 warmup matmuls hold the PE p-state ramp; DMAs are batched into large
    chunks ordered so each step's x / onehot data lands ahead of use.

Each core writes partial tensors; the host combines them into the loss.
"""

import numpy as np

L = 26
D = 128
T = 64
B = 8192
NCORES = 8
BC = B // NCORES  # 1024 sequences per core

# Per-step scale schedule for the exp-space forward DP (subtracted from em at
# step t so the running A stays well inside fp32 range). It cancels exactly
# in the host finale (gold ln-sum and logZ shift by the same B*sum(C_SCHED)).
C_SCHED = np.array([
    0.933700, 3.577268, 3.746262, 4.537820, 4.040299, 4.041378, 4.067604, 4.107736,
    4.101158, 4.091968, 3.790887, 4.203616, 4.050755, 4.272369, 3.625527, 3.864683,
    4.922722, 4.424649, 3.161501, 4.352942, 3.777887, 4.534618, 4.044740, 3.829787,
    4.015547, 4.710327, 3.921810, 4.398400, 4.176108, 3.293104, 4.761852, 3.388780,
    3.782803, 4.950686, 3.611373, 4.506680, 3.005395, 4.511179, 3.714007, 4.567758,
    3.993558, 4.003791, 4.249708, 4.211322, 4.069564, 4.249093, 3.763951, 3.601156,
    5.005219, 3.880518, 4.270474, 3.819207, 3.979380, 4.438228, 4.122883, 2.404448,
    4.026374, 5.060853, 4.290274, 4.044138, 3.681486, 4.656340, 3.408876, 3.532320,
], dtype=np.float64)

_CACHE: dict = {}
TRACE = False  # set by test harness to capture NTFF profile / exec time

# Instruction opcodes whose hardware structs tolerate multiple sync waits (or
# that walrus lowers specially). Everything else gets excess waits peeled onto
# EventSemaphore instructions inserted just before it (same engine).
_MULTIWAIT_OK = {
    "Call",
    "UnconditionalBranch",
    "ConditionalBranch",
}


def _legalize_waits(bir_bytes: bytes) -> bytes:
    """Split >1 sync waits per compute instruction into EventSemaphore preludes.

    The TRN2 64-byte instruction structs hold a single sync-wait command;
    Tile attaches multi-engine waits directly, which walrus codegen rejects
    ("Too many sync wait commands"). Peeling extra waits onto same-engine
    EventSemaphore instructions placed immediately before is semantically
    identical (engine streams execute in order).
    """
    import json

    d = json.loads(bir_bytes)
    n = 0
    for fn in d["functions"]:
        for blk in fn["blocks"]:
            out = []
            for inst in blk["instructions"]:
                si = inst.get("sync_info")
                if (
                    si
                    and len(si.get("on_wait", [])) > 1
                    and inst["opcode"] not in _MULTIWAIT_OK
                ):
                    waits = si["on_wait"]
                    for w in waits[:-1]:
                        n += 1
                        out.append({
                            "debug": inst.get("debug", 0),
                            "engine": inst["engine"],
                            "ins": [],
                            "name": f"wsplit-{n}-{inst['name']}",
                            "opcode": "EventSemaphore",
                            "outs": [],
                            "sync_info": {"on_update": [], "on_wait": [w]},
                        })
                    si["on_wait"] = [waits[-1]]
                out.append(inst)
            blk["instructions"] = out
    return json.dumps(d).encode()


# cblob byte layout (per partition)
_CB_WDR = 0        # [0:64)    Wdr fp8 [64 part, 2, 32]  DoubleRow weights
_CB_WT = 64        # [64:96)   Wt32 fp8 [128, 32]        plain em weights (W^T)
_CB_EXPBD = 96     # [96:352)  expBD bf16 [128, 128]     block-diag exp(Tr)
_CB_CBIAS = 352    # [352:608) cbias f32 [128, 64]       -C_SCHED broadcast
_CB_ONESSH = 608   # [608:1120) onesSh bf16 [128, 8, 32] slot-shifted gather
_CB_ONESBD = 1120  # [1120:1128) onesBD bf16 [128, 4]    group-sum mask
_CB_END = 1128

# out tensor column layout (f32 [128, 2368])
_OUT_S = 0         # rows 0:32, cols [0:2048)   s blocks [32, 8, 256]
_OUT_ZS = 2048     # rows 0:4,  cols [2048:2304) zs [4, 256]
_OUT_CC = 2304     # rows 0:26, cols [2304:2356) CC [26, 52]
_OUT_COLS = 2368


def build_program():
    """Build the per-core Bass/Tile program (identical SPMD program)."""
    from contextlib import ExitStack

    import concourse.bass as bass
    import concourse.tile as tile
    from concourse import mybir

    f32 = mybir.dt.float32
    bf16 = mybir.dt.bfloat16
    f8 = mybir.dt.float8e4
    AF = mybir.ActivationFunctionType
    OP = mybir.AluOpType
    DR = mybir.MatmulPerfMode.DoubleRow

    nc = bass.Bass("TRN2", target_bir_lowering=False, debug=False)

    xa_d = nc.dram_tensor("xa", [64, 2, T, 256], f8, kind="ExternalInput").ap()
    xb_d = nc.dram_tensor("xb", [D, T, 768], f8, kind="ExternalInput").ap()
    oht_d = nc.dram_tensor("oht", [128, T, 256], f8, kind="ExternalInput").ap()
    ohdr_d = nc.dram_tensor("ohdr", [64, 2, T, 8, L], f8, kind="ExternalInput").ap()
    c_d = nc.dram_tensor("cst", [128, _CB_END], mybir.dt.uint8, kind="ExternalInput").ap()
    out_d = nc.dram_tensor("out", [128, _OUT_COLS], f32, kind="ExternalOutput").ap()

    from concourse.tile import add_dep_helper

    # Total-order the PE instruction stream in program order: the greedy Tile
    # scheduler otherwise slots em/gather matmuls ahead of the next step's DP
    # matmul whenever the DP's input isn't ready yet in its internal sim,
    # which threads the serial DP->DVE chain through a step's worth of PE
    # work (in-order engine streams) and inflates the critical cycle.
    _pe_prev = [None]

    def pe_mm(*args, **kwargs):
        mi = nc.tensor.matmul(*args, **kwargs)
        if _pe_prev[0] is not None:
            add_dep_helper(mi.ins, _pe_prev[0].ins, reason="pe-order")
        _pe_prev[0] = mi
        return mi

    with ExitStack() as ctx:
        tc = ctx.enter_context(tile.TileContext(nc))

        const = ctx.enter_context(tc.tile_pool(name="const", bufs=1))
        epool = ctx.enter_context(tc.tile_pool(name="epool", bufs=5))
        apool = ctx.enter_context(tc.tile_pool(name="apool", bufs=2))
        ppool = ctx.enter_context(tc.tile_pool(name="ppool", bufs=3))
        fpool = ctx.enter_context(tc.tile_pool(name="fpool", bufs=1))
        ps_em = ctx.enter_context(tc.tile_pool(name="ps_em", bufs=2, space="PSUM"))
        ps_u1 = ctx.enter_context(tc.tile_pool(name="ps_u1", bufs=1, space="PSUM"))
        ps_u2 = ctx.enter_context(tc.tile_pool(name="ps_u2", bufs=1, space="PSUM"))
        ps_sg = ctx.enter_context(tc.tile_pool(name="ps_sg", bufs=2, space="PSUM"))
        ps_cc = ctx.enter_context(tc.tile_pool(name="ps_cc", bufs=1, space="PSUM"))

        # ---- PE p-state warmup: dummy matmuls keep the tensor engine's
        # ramp running so the first real emissions hit full clock
        wz = const.tile([128, 256], bf16)
        nc.vector.memset(wz, 0.0)
        for w in range(5):
            wps = ps_em.tile([128, 256], f32, tag="em", name="warm")
            pe_mm(
                wps, lhsT=wz[:, 0:128], rhs=wz[:, 0:256], start=True, stop=True
            )

        # ---- SBUF input tiles ----
        xa = const.tile([64, 2, T, 256], f8)
        xb = const.tile([D, T, 768], f8)
        oht = const.tile([128, T, 256], f8)
        ohdr = const.tile([64, 2, T, 8, L], f8)
        cblob = const.tile([128, _CB_END], mybir.dt.uint8)
        fin = fpool.tile([32, _OUT_COLS], f32)

        # packed constants first: single small DMA gates everything
        nc.scalar.dma_start(out=cblob, in_=c_d)

        def dma_xa(t0, t1):
            nc.sync.dma_start(out=xa[:, :, t0:t1, :], in_=xa_d[:, :, t0:t1, :])

        def dma_xb(t0, t1):
            nc.sync.dma_start(out=xb[:, t0:t1, :], in_=xb_d[:, t0:t1, :])

        def dma_oht(t0, t1):
            nc.sync.dma_start(out=oht[:, t0:t1, :], in_=oht_d[:, t0:t1, :])

        def dma_ohdr(t0, t1):
            nc.sync.dma_start(
                out=ohdr[:, :, t0:t1, :, :], in_=ohdr_d[:, :, t0:t1, :, :]
            )

        # front-load tiny first chunks (em(0)/em(1) gate the chain start,
        # and every consumer pays the 900ns DMA-sem propagation), then
        # stream progressively larger chunks ordered several steps ahead
        # of first use; x (chain-critical) leads, oht/ohdr interleave
        dma_xa(0, 2)
        dma_xb(0, 2)
        dma_oht(0, 2)
        dma_xa(2, 5)
        dma_xb(2, 5)
        dma_oht(2, 8)
        dma_ohdr(0, 8)
        dma_xa(5, 9)
        dma_xb(5, 9)
        dma_xa(9, 15)
        dma_xb(9, 15)
        dma_oht(8, 18)
        dma_ohdr(8, 18)
        dma_xa(15, 24)
        dma_xb(15, 24)
        dma_oht(18, 32)
        dma_ohdr(18, 32)
        dma_xa(24, 36)
        dma_xb(24, 36)
        dma_oht(32, 48)
        dma_ohdr(32, 48)
        dma_xa(36, 50)
        dma_xb(36, 50)
        dma_oht(48, 64)
        dma_ohdr(48, 64)
        dma_xa(50, 64)
        dma_xb(50, 64)

        # ---- bitcast views into the packed constant blob ----
        Wdr = cblob[0:64, _CB_WDR : _CB_WDR + 64].bitcast(f8).rearrange(
            "p (j m) -> p j m", j=2
        )
        Wt32 = cblob[:, _CB_WT : _CB_WT + 32].bitcast(f8)
        expBD = cblob[:, _CB_EXPBD : _CB_EXPBD + 256].bitcast(bf16)
        cbias = cblob[:, _CB_CBIAS : _CB_CBIAS + 256].bitcast(f32)
        onesSh = cblob[:, _CB_ONESSH : _CB_ONESSH + 512].bitcast(bf16).rearrange(
            "p (s m) -> p s m", s=8
        )
        onesBD = cblob[:, _CB_ONESBD : _CB_ONESBD + 8].bitcast(bf16)

        # persistent psum accumulator for paired transition counts
        CC_ps = ps_cc.tile([L, 2 * L], f32)
        nc.vector.memset(CC_ps, 0.0)

        E_t = {}
        em_t = {}

        def emit_em_a(t):
            # group 0 (fp8 DoubleRow; only legal at psum partition 0) and
            # groups 1-2 plain fp8
            em_ps = ps_em.tile([128, 256], f32, tag="em")
            em_t[t] = em_ps
            pe_mm(
                em_ps[0:32, :],
                lhsT=Wdr,
                rhs=xa[:, :, t, :],
                start=True,
                stop=True,
                perf_mode=DR,
                tile_position=(0, 0),
            )
            for g in (1, 2):
                pe_mm(
                    em_ps[32 * g : 32 * (g + 1), :],
                    lhsT=Wt32,
                    rhs=xb[:, t, 256 * (g - 1) : 256 * g],
                    start=True,
                    stop=True,
                    tile_position=(0, 32 * g),
                )

        def emit_em_b(t):
            # group 3 plain fp8 (placed after the step's second DP half)
            pe_mm(
                em_t[t][96:128, :],
                lhsT=Wt32,
                rhs=xb[:, t, 512:768],
                start=True,
                stop=True,
                tile_position=(0, 96),
            )

        def emit_em(t):
            emit_em_a(t)
            emit_em_b(t)

        def emit_exp(t):
            E = epool.tile([128, 256], bf16, tag="E", name="E")
            nc.scalar.activation(
                E, em_t.pop(t), AF.Exp, bias=cbias[:, t : t + 1], scale=1.0
            )
            E_t[t] = E

        sg_tiles = {}

        P_t = {}

        def emit_P(t):
            # P_t = OHT_t o E_t on Pool (the only PSUM-free elementwise
            # engine); issued one step ahead of the gather so the ~600ns
            # Pool op stays off the PE queue's critical path
            P = ppool.tile([128, 256], bf16, tag="P", name="P")
            nc.gpsimd.tensor_tensor(out=P, in0=oht[:, t, :], in1=E_t[t], op=OP.mult)
            P_t[t] = P

        def emit_gather(t):
            # slot-shifted ones-gather accumulates E_t[y_t, b] into psum
            s, q = t % 8, t // 8
            if s == 0:
                sg_tiles[q % 2] = ps_sg.tile([32, 256], f32, tag="sg", name="sg")
            pe_mm(
                sg_tiles[q % 2],
                lhsT=onesSh[:, s, :],
                rhs=P_t.pop(t),
                start=(s == 0),
                stop=(s == 7),
            )

        def emit_scopy(q):
            nc.scalar.copy(fin[:, 256 * q : 256 * (q + 1)], sg_tiles[q % 2])
            if q % 2 == 1 and q < 7:
                nc.sync.dma_start(
                    out=out_d[0:32, 256 * (q - 1) : 256 * (q + 1)],
                    in_=fin[:, 256 * (q - 1) : 256 * (q + 1)],
                )

        def emit_counts(a, chunks=range(8)):
            # paired transition counts, anchor a (even): one fp8 DoubleRow
            # matmul per b-chunk covers pairs (a-1,a) [transposed] and (a,a+1)
            for c in chunks:
                lhsT = ohdr[:, :, a, c, :]
                if a == 0:
                    rhs = ohdr[:, :, 1:2, c, :]
                    outap = CC_ps[:, L : 2 * L]
                else:
                    rhs = ohdr[:, :, a - 1 : a + 2 : 2, c, :]
                    outap = CC_ps
                pe_mm(
                    outap,
                    lhsT=lhsT,
                    rhs=rhs,
                    start=False,
                    stop=False,
                    perf_mode=DR,
                    skip_group_check=True,
                )

        # ---- software-pipelined main loop ----
        emit_em(0)
        emit_exp(0)
        emit_em(1)
        emit_exp(1)
        emit_P(0)
        A_prev = None
        for t in range(T):
            E = E_t[t]
            if t == 0:
                A_prev = E
                if t + 2 < T:
                    emit_em(t + 2)
                    emit_exp(t + 2)
            else:
                # chain halves: DP-H2 is deliberately placed after two em
                # matmuls so its consumer (the DVE-serialized second A-half)
                # is never the critical edge; DP-H1(t+1) only needs A-H1(t).
                with tc.high_priority(offset=60):
                    u1 = ps_u1.tile([128, 128], f32, tag="u1", name="u1")
                    A_new = apool.tile([128, 256], bf16, tag="A", name="A")
                    pe_mm(u1, lhsT=expBD, rhs=A_prev[:, 0:128], start=True, stop=True)
                    nc.vector.tensor_mul(A_new[:, 0:128], u1, E[:, 0:128])
                if t + 2 < T:
                    emit_em_a(t + 2)
                with tc.high_priority(offset=60):
                    u2 = ps_u2.tile([128, 128], f32, tag="u2", name="u2")
                    pe_mm(u2, lhsT=expBD, rhs=A_prev[:, 128:256], start=True, stop=True)
                    nc.vector.tensor_mul(A_new[:, 128:256], u2, E[:, 128:256])
                if t + 2 < T:
                    emit_em_b(t + 2)
                    emit_exp(t + 2)
                A_prev = A_new
            if t + 1 < T:
                emit_P(t + 1)
            emit_gather(t)
            E_t.pop(t)
            if t % 8 == 7:
                emit_scopy(t // 8)
            if t >= 2 and t % 2 == 0:
                emit_counts(t - 2, range(4))
            elif t >= 3:
                emit_counts(t - 3, range(4, 8))
        emit_counts(T - 2)

        # ---- finale: stage zs/CC into fin; copies parallel on DVE/Act ----
        zs_ps = ps_em.tile([4, 256], f32, tag="em", name="zs")
        pe_mm(zs_ps, lhsT=onesBD, rhs=A_prev, start=True, stop=True)
        nc.vector.tensor_copy(fin[0:L, _OUT_CC : _OUT_CC + 2 * L], CC_ps)
        nc.scalar.copy(fin[0:4, _OUT_ZS : _OUT_ZS + 256], zs_ps)
        nc.sync.dma_start(
            out=out_d[0:32, 1536:_OUT_COLS], in_=fin[:, 1536:_OUT_COLS]
        )

    fixed = _legalize_waits(nc.to_json_bytes())
    nc.to_json_bytes = lambda: fixed  # shadow for all compile paths
    return nc


def _marshal(feat_x, input_y, params):
    """Host-side input marshalling: dtype casts + layout transposes/onehots."""
    import ml_dtypes

    f8 = ml_dtypes.float8_e4m3
    bf16 = ml_dtypes.bfloat16

    feat_x = np.asarray(feat_x, dtype=np.float32)
    input_y = np.asarray(input_y, dtype=np.int32)
    params = np.asarray(params, dtype=np.float32)

    W = params[: L * D].reshape(L, D)
    Tr = params[L * D :].reshape(L, L).astype(np.float64)

    # ---- packed per-partition constants ----
    cblob = np.zeros((128, _CB_END), dtype=np.uint8)
    # Wdr [64, 2, 32]: Wdr[k, j, m] = W[m, 2k+j]
    wdr = np.zeros((64, 2, 32), dtype=np.float32)
    wdr[:, :, :L] = W.T.reshape(64, 2, L)
    cblob[0:64, _CB_WDR : _CB_WDR + 64] = (
        wdr.astype(f8).view(np.uint8).reshape(64, 64)
    )
    # Wt32 [128, 32]: W^T zero-padded
    wt32 = np.zeros((D, 32), dtype=np.float32)
    wt32[:, :L] = W.T
    cblob[:, _CB_WT : _CB_WT + 32] = wt32.astype(f8).view(np.uint8)
    # expBD block-diag exp(Tr)
    expbd = np.zeros((128, 128), dtype=np.float32)
    for g in range(4):
        expbd[32 * g : 32 * g + L, 32 * g : 32 * g + L] = np.exp(Tr)
    cblob[:, _CB_EXPBD : _CB_EXPBD + 256] = expbd.astype(bf16).view(np.uint8)
    # cbias
    cbias = np.tile(-C_SCHED.astype(np.float32), (128, 1))
    cblob[:, _CB_CBIAS : _CB_CBIAS + 256] = cbias.view(np.uint8)
    # onesSh [128, 8, 32]: onesSh[32g+l, s, 4s+g] = 1 for l < L
    onessh = np.zeros((128, 8, 32), dtype=np.float32)
    for g in range(4):
        for s in range(8):
            onessh[32 * g : 32 * g + L, s, 4 * s + g] = 1.0
    cblob[:, _CB_ONESSH : _CB_ONESSH + 512] = (
        onessh.astype(bf16).view(np.uint8).reshape(128, 512)
    )
    # onesBD [128, 4]
    onesbd = np.zeros((128, 4), dtype=np.float32)
    for g in range(4):
        onesbd[32 * g : 32 * g + L, g] = 1.0
    cblob[:, _CB_ONESBD : _CB_ONESBD + 8] = onesbd.astype(bf16).view(np.uint8)
    cblob = np.ascontiguousarray(cblob)

    # x transposed once: xT[d, t, b]
    xT = np.ascontiguousarray(feat_x.transpose(2, 1, 0)).astype(f8)

    in_maps = []
    for m in range(NCORES):
        sl = slice(m * BC, (m + 1) * BC)
        xm = xT[:, :, sl]  # [128, T, 1024] fp8
        ym = input_y[sl]  # [1024, T]
        # xa [64, 2, T, 256]: group 0 (b 0:256), d = 2k+j
        xam = np.ascontiguousarray(
            xm[:, :, 0:256].reshape(64, 2, T, 256)
        )
        # xb [128, T, 768]: groups 1-3 (b 256:1024)
        xbm = np.ascontiguousarray(xm[:, :, 256:1024])
        # oht [128, T, 256]: 448 where (y[256g+b', t] == l) else 0 - the
        # device masks via min(oht, E) on the Pool engine
        lab = ym.reshape(4, 256, T)  # [g, b', t]
        lvec = np.arange(32)
        ohtm = (
            lab[:, None, :, :] == lvec[None, :, None, None]
        )  # [g, l(32), b', t]
        ohtm = np.ascontiguousarray(
            ohtm.reshape(128, 256, T).transpose(0, 2, 1).astype(np.float32)
        ).astype(f8)
        # ohdr [64, 2, T, 8, L]: ohdr[k, j, t, c, l] = (y[128c+2k+j, t] == l)
        labc = ym.reshape(8, 64, 2, T)  # [c, k, j, t]
        ohdrm = labc[:, :, :, :, None] == np.arange(L)[None, None, None, None, :]
        ohdrm = np.ascontiguousarray(
            ohdrm.transpose(1, 2, 3, 0, 4).astype(np.float32)
        ).astype(f8)
        in_maps.append(
            {"xa": xam, "xb": xbm, "oht": ohtm, "ohdr": ohdrm, "cst": cblob}
        )
    return in_maps


def kernel(feat_x: np.ndarray, input_y: np.ndarray, params: np.ndarray) -> np.ndarray:
    from concourse.bass_utils import run_bass_kernel_spmd

    if "nc" not in _CACHE:
        _CACHE["nc"] = build_program()
    nc = _CACHE["nc"]

    in_maps = _marshal(feat_x, input_y, params)

    res = run_bass_kernel_spmd(
        nc, in_maps, core_ids=list(range(NCORES)), trace=TRACE
    )
    _CACHE["last_results"] = res

    params = np.asarray(params, dtype=np.float64)
    Tr = params[L * D :].reshape(L, L)

    lns_sum = lnz_sum = tr_sum = 0.0
    for m in range(NCORES):
        out = res.results[m]["out"].astype(np.float64)
        s = out[0:32, _OUT_S : _OUT_S + 2048]
        lns_sum += np.log(np.maximum(s, 1e-300)).sum()
        zs = out[0:4, _OUT_ZS : _OUT_ZS + 256]
        lnz_sum += np.log(zs).sum()
        cc = out[0:L, _OUT_CC : _OUT_CC + 2 * L]
        tr_sum += (Tr.T * cc[:, 0:L]).sum() + (Tr * cc[:, L : 2 * L]).sum()
    loss = -(lns_sum + tr_sum - lnz_sum) / B
    return np.float32(loss)


# revision 45
# speedup vs baseline: 1.3820x; 1.0293x over previous
"""Linear-chain CRF negative mean log-likelihood on 8 Trainium2 NeuronCores.

Full inputs in, full (scalar) output out. Data-parallel over the batch: each
core processes B/8 = 1024 sequences end-to-end.

v3 architecture (per core), engine-balanced around the serial forward-DP:
  - emission scores em[32g+l, b'] per step via 4 matmuls: group 0 rides the
    fp8 DoubleRow perf mode (0.5 cy/col; only legal at PSUM partition 0),
    groups 1-3 plain fp8. x is host-marshalled into the matching layouts.
  - partition function via the exp-space forward DP
    A_t = (expBD^T A_{t-1}) o exp(em_t - c_t), split into two 128-column
    half-chains so the PE->DVE->PE dependency cycle (53 + ~104 + 258 + ~81 ns)
    stays under the engine budgets. The DVE runs ONLY these two A-multiplies
    per step - all gold-score work is off the DVE.
  - gold emission score: Pool computes P_t = OHT_t o E_t (the only
    PSUM-free elementwise engine); a ones-gather matmul with a slot-shifted
    stationary accumulates s[4s+g, b'] = E_t[y_t, b] into a rotating PSUM
    block (8 steps per block); Act copies blocks out and the host does the
    ln + sum (the c-schedule cancels against logZ).
  - gold transition score: paired count matmuls anchored at even t with
    moving [oh_{t-1} | oh_{t+1}] in fp8 DoubleRow (26 cy each, 4/step avg)
    accumulate [l_t, l_prev | l_next] counts; host contracts with Tr^T | Tr.
  - logZ: group sums zs = onesBD^T A_63 shipped raw; host ln + reduce.
  - 8 warmup matmuls hold the PE p-state ramp; DMAs are batched into large
    chunks ordered so each step's x / onehot data lands ahead of use.

Each core writes partial tensors; the host combines them into the loss.
"""

import numpy as np

L = 26
D = 128
T = 64
B = 8192
NCORES = 8
BC = B // NCORES  # 1024 sequences per core

# Per-step scale schedule for the exp-space forward DP (subtracted from em at
# step t so the running A stays well inside fp32 range). It cancels exactly
# in the host finale (gold ln-sum and logZ shift by the same B*sum(C_SCHED)).
C_SCHED = np.array([
    0.933700, 3.577268, 3.746262, 4.537820, 4.040299, 4.041378, 4.067604, 4.107736,
    4.101158, 4.091968, 3.790887, 4.203616, 4.050755, 4.272369, 3.625527, 3.864683,
    4.922722, 4.424649, 3.161501, 4.352942, 3.777887, 4.534618, 4.044740, 3.829787,
    4.015547, 4.710327, 3.921810, 4.398400, 4.176108, 3.293104, 4.761852, 3.388780,
    3.782803, 4.950686, 3.611373, 4.506680, 3.005395, 4.511179, 3.714007, 4.567758,
    3.993558, 4.003791, 4.249708, 4.211322, 4.069564, 4.249093, 3.763951, 3.601156,
    5.005219, 3.880518, 4.270474, 3.819207, 3.979380, 4.438228, 4.122883, 2.404448,
    4.026374, 5.060853, 4.290274, 4.044138, 3.681486, 4.656340, 3.408876, 3.532320,
], dtype=np.float64)

_CACHE: dict = {}
TRACE = False  # set by test harness to capture NTFF profile / exec time

# Instruction opcodes whose hardware structs tolerate multiple sync waits (or
# that walrus lowers specially). Everything else gets excess waits peeled onto
# EventSemaphore instructions inserted just before it (same engine).
_MULTIWAIT_OK = {
    "Call",
    "UnconditionalBranch",
    "ConditionalBranch",
}


def _legalize_waits(bir_bytes: bytes) -> bytes:
    """Split >1 sync waits per compute instruction into EventSemaphore preludes.

    The TRN2 64-byte instruction structs hold a single sync-wait command;
    Tile attaches multi-engine waits directly, which walrus codegen rejects
    ("Too many sync wait commands"). Peeling extra waits onto same-engine
    EventSemaphore instructions placed immediately before is semantically
    identical (engine streams execute in order).
    """
    import json

    d = json.loads(bir_bytes)
    n = 0
    for fn in d["functions"]:
        for blk in fn["blocks"]:
            out = []
            for inst in blk["instructions"]:
                si = inst.get("sync_info")
                if (
                    si
                    and len(si.get("on_wait", [])) > 1
                    and inst["opcode"] not in _MULTIWAIT_OK
                ):
                    waits = si["on_wait"]
                    for w in waits[:-1]:
                        n += 1
                        out.append({
                            "debug": inst.get("debug", 0),
                            "engine": inst["engine"],
                            "ins": [],
                            "name": f"wsplit-{n}-{inst['name']}",
                            "opcode": "EventSemaphore",
                            "outs": [],
                            "sync_info": {"on_update": [], "on_wait": [w]},
                        })
                    si["on_wait"] = [waits[-1]]
                out.append(inst)
            blk["instructions"] = out
    return json.dumps(d).encode()


# cblob byte layout (per partition)
_CB_WDR = 0        # [0:64)    Wdr fp8 [64 part, 2, 32]  DoubleRow weights
_CB_WT = 64        # [64:96)   Wt32 fp8 [128, 32]        plain em weights (W^T)
_CB_EXPBD = 96     # [96:352)  expBD bf16 [128, 128]     block-diag exp(Tr)
_CB_CBIAS = 352    # [352:608) cbias f32 [128, 64]       -C_SCHED broadcast
_CB_ONESSH = 608   # [608:1120) onesSh bf16 [128, 8, 32] slot-shifted gather
_CB_ONESBD = 1120  # [1120:1128) onesBD bf16 [128, 4]    group-sum mask
_CB_END = 1128

# out tensor column layout (f32 [128, 2368])
_OUT_S = 0         # rows 0:32, cols [0:2048)   s blocks [32, 8, 256]
_OUT_A = 2048      # rows 0:4,  cols [2048:2304) zs [4, 256]
_OUT_CC = 2304     # rows 0:26, cols [2304:2356) CC [26, 52]
_OUT_COLS = 2368


def build_program():
    """Build the per-core Bass/Tile program (identical SPMD program)."""
    from contextlib import ExitStack

    import concourse.bass as bass
    import concourse.tile as tile
    from concourse import mybir

    f32 = mybir.dt.float32
    bf16 = mybir.dt.bfloat16
    f8 = mybir.dt.float8e4
    AF = mybir.ActivationFunctionType
    OP = mybir.AluOpType
    DR = mybir.MatmulPerfMode.DoubleRow

    nc = bass.Bass("TRN2", target_bir_lowering=False, debug=False)

    xa_d = nc.dram_tensor("xa", [64, 2, T, 256], f8, kind="ExternalInput").ap()
    xb_d = nc.dram_tensor("xb", [D, T, 768], f8, kind="ExternalInput").ap()
    oht_d = nc.dram_tensor("oht", [128, T, 256], f8, kind="ExternalInput").ap()
    ohdr_d = nc.dram_tensor("ohdr", [64, 2, T, 8, L], f8, kind="ExternalInput").ap()
    c_d = nc.dram_tensor("cst", [128, _CB_END], mybir.dt.uint8, kind="ExternalInput").ap()
    out_d = nc.dram_tensor("out", [128, _OUT_COLS], f32, kind="ExternalOutput").ap()

    from concourse.tile import add_dep_helper

    # Total-order the PE instruction stream in program order: the greedy Tile
    # scheduler otherwise slots em/gather matmuls ahead of the next step's DP
    # matmul whenever the DP's input isn't ready yet in its internal sim,
    # which threads the serial DP->DVE chain through a step's worth of PE
    # work (in-order engine streams) and inflates the critical cycle.
    _pe_prev = [None]

    def pe_mm(*args, **kwargs):
        mi = nc.tensor.matmul(*args, **kwargs)
        if _pe_prev[0] is not None:
            add_dep_helper(mi.ins, _pe_prev[0].ins, reason="pe-order")
        _pe_prev[0] = mi
        return mi

    with ExitStack() as ctx:
        tc = ctx.enter_context(tile.TileContext(nc))

        const = ctx.enter_context(tc.tile_pool(name="const", bufs=1))
        epool = ctx.enter_context(tc.tile_pool(name="epool", bufs=5))
        apool = ctx.enter_context(tc.tile_pool(name="apool", bufs=2))
        ppool = ctx.enter_context(tc.tile_pool(name="ppool", bufs=3))
        fpool = ctx.enter_context(tc.tile_pool(name="fpool", bufs=1))
        ps_em = ctx.enter_context(tc.tile_pool(name="ps_em", bufs=2, space="PSUM"))
        ps_u1 = ctx.enter_context(tc.tile_pool(name="ps_u1", bufs=1, space="PSUM"))
        ps_u2 = ctx.enter_context(tc.tile_pool(name="ps_u2", bufs=1, space="PSUM"))
        ps_sg = ctx.enter_context(tc.tile_pool(name="ps_sg", bufs=2, space="PSUM"))
        ps_cc = ctx.enter_context(tc.tile_pool(name="ps_cc", bufs=1, space="PSUM"))

        # ---- PE p-state warmup: dummy matmuls keep the tensor engine's
        # ramp running so the first real emissions hit full clock
        wz = const.tile([128, 256], bf16)
        nc.vector.memset(wz, 0.0)
        for w in range(5):
            wps = ps_em.tile([128, 256], f32, tag="em", name="warm")
            pe_mm(
                wps, lhsT=wz[:, 0:128], rhs=wz[:, 0:256], start=True, stop=True
            )

        # ---- SBUF input tiles ----
        xa = const.tile([64, 2, T, 256], f8)
        xb = const.tile([D, T, 768], f8)
        oht = const.tile([128, T, 256], f8)
        ohdr = const.tile([64, 2, T, 8, L], f8)
        cblob = const.tile([128, _CB_END], mybir.dt.uint8)
        fin = fpool.tile([32, _OUT_COLS], f32)

        # packed constants first: single small DMA gates everything
        nc.scalar.dma_start(out=cblob, in_=c_d)

        def dma_xa(t0, t1):
            nc.sync.dma_start(out=xa[:, :, t0:t1, :], in_=xa_d[:, :, t0:t1, :])

        def dma_xb(t0, t1):
            nc.sync.dma_start(out=xb[:, t0:t1, :], in_=xb_d[:, t0:t1, :])

        def dma_oht(t0, t1):
            nc.sync.dma_start(out=oht[:, t0:t1, :], in_=oht_d[:, t0:t1, :])

        def dma_ohdr(t0, t1):
            nc.sync.dma_start(
                out=ohdr[:, :, t0:t1, :, :], in_=ohdr_d[:, :, t0:t1, :, :]
            )

        # front-load tiny first chunks (em(0)/em(1) gate the chain start,
        # and every consumer pays the 900ns DMA-sem propagation), then
        # stream progressively larger chunks ordered several steps ahead
        # of first use; x (chain-critical) leads, oht/ohdr interleave
        dma_xa(0, 2)
        dma_xb(0, 2)
        dma_oht(0, 2)
        dma_xa(2, 5)
        dma_xb(2, 5)
        dma_oht(2, 8)
        dma_ohdr(0, 8)
        dma_xa(5, 9)
        dma_xb(5, 9)
        dma_xa(9, 15)
        dma_xb(9, 15)
        dma_oht(8, 18)
        dma_ohdr(8, 18)
        dma_xa(15, 24)
        dma_xb(15, 24)
        dma_oht(18, 32)
        dma_ohdr(18, 32)
        dma_xa(24, 36)
        dma_xb(24, 36)
        dma_oht(32, 48)
        dma_ohdr(32, 48)
        dma_xa(36, 50)
        dma_xb(36, 50)
        dma_oht(48, 64)
        dma_ohdr(48, 64)
        dma_xa(50, 64)
        dma_xb(50, 64)

        # ---- bitcast views into the packed constant blob ----
        Wdr = cblob[0:64, _CB_WDR : _CB_WDR + 64].bitcast(f8).rearrange(
            "p (j m) -> p j m", j=2
        )
        Wt32 = cblob[:, _CB_WT : _CB_WT + 32].bitcast(f8)
        expBD = cblob[:, _CB_EXPBD : _CB_EXPBD + 256].bitcast(bf16)
        cbias = cblob[:, _CB_CBIAS : _CB_CBIAS + 256].bitcast(f32)
        onesSh = cblob[:, _CB_ONESSH : _CB_ONESSH + 512].bitcast(bf16).rearrange(
            "p (s m) -> p s m", s=8
        )
        onesBD = cblob[:, _CB_ONESBD : _CB_ONESBD + 8].bitcast(bf16)

        # persistent psum accumulator for paired transition counts
        CC_ps = ps_cc.tile([L, 2 * L], f32)
        nc.vector.memset(CC_ps, 0.0)

        E_t = {}
        em_t = {}

        def emit_em_a(t):
            # group 0 (fp8 DoubleRow; only legal at psum partition 0) and
            # groups 1-2 plain fp8
            em_ps = ps_em.tile([128, 256], f32, tag="em")
            em_t[t] = em_ps
            pe_mm(
                em_ps[0:32, :],
                lhsT=Wdr,
                rhs=xa[:, :, t, :],
                start=True,
                stop=True,
                perf_mode=DR,
                tile_position=(0, 0),
            )
            for g in (1, 2):
                pe_mm(
                    em_ps[32 * g : 32 * (g + 1), :],
                    lhsT=Wt32,
                    rhs=xb[:, t, 256 * (g - 1) : 256 * g],
                    start=True,
                    stop=True,
                    tile_position=(0, 32 * g),
                )

        def emit_em_b(t):
            # group 3 plain fp8 (placed after the step's second DP half)
            pe_mm(
                em_t[t][96:128, :],
                lhsT=Wt32,
                rhs=xb[:, t, 512:768],
                start=True,
                stop=True,
                tile_position=(0, 96),
            )

        def emit_em(t):
            emit_em_a(t)
            emit_em_b(t)

        def emit_exp(t):
            E = epool.tile([128, 256], bf16, tag="E", name="E")
            nc.scalar.activation(
                E, em_t.pop(t), AF.Exp, bias=cbias[:, t : t + 1], scale=1.0
            )
            E_t[t] = E

        sg_tiles = {}

        P_t = {}

        def emit_P(t):
            # P_t = OHT_t o E_t on Pool (the only PSUM-free elementwise
            # engine); issued one step ahead of the gather so the ~600ns
            # Pool op stays off the PE queue's critical path
            P = ppool.tile([128, 256], bf16, tag="P", name="P")
            nc.gpsimd.tensor_tensor(out=P, in0=oht[:, t, :], in1=E_t[t], op=OP.mult)
            P_t[t] = P

        def emit_gather(t):
            # slot-shifted ones-gather accumulates E_t[y_t, b] into psum
            s, q = t % 8, t // 8
            if s == 0:
                sg_tiles[q % 2] = ps_sg.tile([32, 256], f32, tag="sg", name="sg")
            pe_mm(
                sg_tiles[q % 2],
                lhsT=onesSh[:, s, :],
                rhs=P_t.pop(t),
                start=(s == 0),
                stop=(s == 7),
            )

        def emit_scopy(q):
            nc.scalar.copy(fin[:, 256 * q : 256 * (q + 1)], sg_tiles[q % 2])
            if q % 2 == 1 and q < 7:
                nc.sync.dma_start(
                    out=out_d[0:32, 256 * (q - 1) : 256 * (q + 1)],
                    in_=fin[:, 256 * (q - 1) : 256 * (q + 1)],
                )

        def emit_counts(a, chunks=range(8)):
            # paired transition counts, anchor a (even): one fp8 DoubleRow
            # matmul per b-chunk covers pairs (a-1,a) [transposed] and (a,a+1)
            for c in chunks:
                lhsT = ohdr[:, :, a, c, :]
                if a == 0:
                    rhs = ohdr[:, :, 1:2, c, :]
                    outap = CC_ps[:, L : 2 * L]
                else:
                    rhs = ohdr[:, :, a - 1 : a + 2 : 2, c, :]
                    outap = CC_ps
                pe_mm(
                    outap,
                    lhsT=lhsT,
                    rhs=rhs,
                    start=False,
                    stop=False,
                    perf_mode=DR,
                    skip_group_check=True,
                )

        # ---- software-pipelined main loop ----
        emit_em(0)
        emit_exp(0)
        emit_em(1)
        emit_exp(1)
        emit_P(0)
        A_prev = None
        for t in range(T):
            E = E_t[t]
            if t == 0:
                A_prev = E
            else:
                # chain halves: in steady state DP-H2 is deliberately placed
                # after two em matmuls so its consumer (the DVE-serialized
                # second A-half) is never the critical edge; in the DMA-gated
                # first steps the ems go last so a late x chunk can never
                # block the chain head in the in-order PE stream.
                early = t < 7
                with tc.high_priority(offset=60):
                    u1 = ps_u1.tile([128, 128], f32, tag="u1", name="u1")
                    A_new = apool.tile([128, 256], bf16, tag="A", name="A")
                    pe_mm(u1, lhsT=expBD, rhs=A_prev[:, 0:128], start=True, stop=True)
                    nc.vector.tensor_mul(A_new[:, 0:128], u1, E[:, 0:128])
                if t + 2 < T and not early:
                    emit_em_a(t + 2)
                with tc.high_priority(offset=60):
                    u2 = ps_u2.tile([128, 128], f32, tag="u2", name="u2")
                    pe_mm(u2, lhsT=expBD, rhs=A_prev[:, 128:256], start=True, stop=True)
                    nc.vector.tensor_mul(A_new[:, 128:256], u2, E[:, 128:256])
                if t == 1:
                    emit_em(2)
                    emit_exp(2)
                if t + 2 < T:
                    if early:
                        emit_em_a(t + 2)
                    emit_em_b(t + 2)
                    emit_exp(t + 2)
                A_prev = A_new
            if t + 1 < T:
                emit_P(t + 1)
            if t >= 1:
                # gather runs one iteration behind its P so the ~600ns Pool
                # op always has >1 period of slack before its PE consumer
                emit_gather(t - 1)
                if t % 8 == 0:
                    emit_scopy(t // 8 - 1)
            E_t.pop(t)
            if t >= 2 and t % 2 == 0:
                emit_counts(t - 2, range(4))
            elif t >= 3:
                emit_counts(t - 3, range(4, 8))
        emit_gather(T - 1)
        emit_scopy(7)
        emit_counts(T - 2)

        # ---- finale: stage zs/CC into fin; copies parallel on DVE/Act ----
        zs_ps = ps_em.tile([4, 256], f32, tag="em", name="zs")
        pe_mm(zs_ps, lhsT=onesBD, rhs=A_prev, start=True, stop=True)
        nc.vector.tensor_copy(fin[0:L, _OUT_CC : _OUT_CC + 2 * L], CC_ps)
        nc.scalar.copy(fin[0:4, _OUT_A : _OUT_A + 256], zs_ps)
        nc.sync.dma_start(
            out=out_d[0:32, 1536:_OUT_COLS], in_=fin[:, 1536:_OUT_COLS]
        )

    fixed = _legalize_waits(nc.to_json_bytes())
    nc.to_json_bytes = lambda: fixed  # shadow for all compile paths
    return nc


def _marshal(feat_x, input_y, params):
    """Host-side input marshalling: dtype casts + layout transposes/onehots."""
    import ml_dtypes

    f8 = ml_dtypes.float8_e4m3
    bf16 = ml_dtypes.bfloat16

    feat_x = np.asarray(feat_x, dtype=np.float32)
    input_y = np.asarray(input_y, dtype=np.int32)
    params = np.asarray(params, dtype=np.float32)

    W = params[: L * D].reshape(L, D)
    Tr = params[L * D :].reshape(L, L).astype(np.float64)

    # ---- packed per-partition constants ----
    cblob = np.zeros((128, _CB_END), dtype=np.uint8)
    # Wdr [64, 2, 32]: Wdr[k, j, m] = W[m, 2k+j]
    wdr = np.zeros((64, 2, 32), dtype=np.float32)
    wdr[:, :, :L] = W.T.reshape(64, 2, L)
    cblob[0:64, _CB_WDR : _CB_WDR + 64] = (
        wdr.astype(f8).view(np.uint8).reshape(64, 64)
    )
    # Wt32 [128, 32]: W^T zero-padded
    wt32 = np.zeros((D, 32), dtype=np.float32)
    wt32[:, :L] = W.T
    cblob[:, _CB_WT : _CB_WT + 32] = wt32.astype(f8).view(np.uint8)
    # expBD block-diag exp(Tr)
    expbd = np.zeros((128, 128), dtype=np.float32)
    for g in range(4):
        expbd[32 * g : 32 * g + L, 32 * g : 32 * g + L] = np.exp(Tr)
    cblob[:, _CB_EXPBD : _CB_EXPBD + 256] = expbd.astype(bf16).view(np.uint8)
    # cbias
    cbias = np.tile(-C_SCHED.astype(np.float32), (128, 1))
    cblob[:, _CB_CBIAS : _CB_CBIAS + 256] = cbias.view(np.uint8)
    # onesSh [128, 8, 32]: onesSh[32g+l, s, 4s+g] = 1 for l < L
    onessh = np.zeros((128, 8, 32), dtype=np.float32)
    for g in range(4):
        for s in range(8):
            onessh[32 * g : 32 * g + L, s, 4 * s + g] = 1.0
    cblob[:, _CB_ONESSH : _CB_ONESSH + 512] = (
        onessh.astype(bf16).view(np.uint8).reshape(128, 512)
    )
    # onesBD [128, 4]
    onesbd = np.zeros((128, 4), dtype=np.float32)
    for g in range(4):
        onesbd[32 * g : 32 * g + L, g] = 1.0
    cblob[:, _CB_ONESBD : _CB_ONESBD + 8] = onesbd.astype(bf16).view(np.uint8)
    cblob = np.ascontiguousarray(cblob)

    # x transposed once: xT[d, t, b]
    xT = np.ascontiguousarray(feat_x.transpose(2, 1, 0)).astype(f8)

    in_maps = []
    for m in range(NCORES):
        sl = slice(m * BC, (m + 1) * BC)
        xm = xT[:, :, sl]  # [128, T, 1024] fp8
        ym = input_y[sl]  # [1024, T]
        # xa [64, 2, T, 256]: group 0 (b 0:256), d = 2k+j
        xam = np.ascontiguousarray(
            xm[:, :, 0:256].reshape(64, 2, T, 256)
        )
        # xb [128, T, 768]: groups 1-3 (b 256:1024)
        xbm = np.ascontiguousarray(xm[:, :, 256:1024])
        # oht [128, T, 256]: 448 where (y[256g+b', t] == l) else 0 - the
        # device masks via min(oht, E) on the Pool engine
        lab = ym.reshape(4, 256, T)  # [g, b', t]
        lvec = np.arange(32)
        ohtm = (
            lab[:, None, :, :] == lvec[None, :, None, None]
        )  # [g, l(32), b', t]
        ohtm = np.ascontiguousarray(
            ohtm.reshape(128, 256, T).transpose(0, 2, 1).astype(np.float32)
        ).astype(f8)
        # ohdr [64, 2, T, 8, L]: ohdr[k, j, t, c, l] = (y[128c+2k+j, t] == l)
        labc = ym.reshape(8, 64, 2, T)  # [c, k, j, t]
        ohdrm = labc[:, :, :, :, None] == np.arange(L)[None, None, None, None, :]
        ohdrm = np.ascontiguousarray(
            ohdrm.transpose(1, 2, 3, 0, 4).astype(np.float32)
        ).astype(f8)
        in_maps.append(
            {"xa": xam, "xb": xbm, "oht": ohtm, "ohdr": ohdrm, "cst": cblob}
        )
    return in_maps


def kernel(feat_x: np.ndarray, input_y: np.ndarray, params: np.ndarray) -> np.ndarray:
    from concourse.bass_utils import run_bass_kernel_spmd

    if "nc" not in _CACHE:
        _CACHE["nc"] = build_program()
    nc = _CACHE["nc"]

    in_maps = _marshal(feat_x, input_y, params)

    res = run_bass_kernel_spmd(
        nc, in_maps, core_ids=list(range(NCORES)), trace=TRACE
    )
    _CACHE["last_results"] = res

    params = np.asarray(params, dtype=np.float64)
    Tr = params[L * D :].reshape(L, L)

    import ml_dtypes

    lns_sum = lnz_sum = tr_sum = 0.0
    for m in range(NCORES):
        out = res.results[m]["out"]
        s = out[0:32, _OUT_S : _OUT_S + 2048].astype(np.float64)
        lns_sum += np.log(np.maximum(s, 1e-300)).sum()
        zs = out[0:4, _OUT_A : _OUT_A + 256].astype(np.float64)
        lnz_sum += np.log(zs).sum()
        cc = out[0:L, _OUT_CC : _OUT_CC + 2 * L].astype(np.float64)
        tr_sum += (Tr.T * cc[:, 0:L]).sum() + (Tr * cc[:, L : 2 * L]).sum()
    loss = -(lns_sum + tr_sum - lnz_sum) / B
    return np.float32(loss)
